# revision 1
# baseline (speedup 1.0000x reference)
"""nn_C3PartialConv — v6: Q=2 column-shift banding, 504 matmuls (vs 640).

Banded formulation with TWO kernel-column shifts folded into K:
  K = 120 = 10 s-rows x 6 c x 2 q     (q = column shift 0/1)
  M = 96  = 6 output rows i x 16 cout (all couts in one matmul)
  N = 504 = 2 images x 252 cols
  3 dj-group matmuls per PSUM group: d=0 covers dj {0,1}, d=1 {2,3},
  d=2 {4,5} with the dj=5 weights zero (the q=1 column overhang reads a
  host-zeroed pad column, and its weight is zero anyway).
Groups: 42 row-bands (stride 6, exactly covering 252 output rows) x 4
image pairs = 168 groups; slabs of <=8 banks; dj-group-outer loop with a
scheduler-only barrier per slab keeps long same-engine runs.
"""

import os
import numpy as np
import ml_dtypes

import concourse.bass as bass
import concourse.tile as tile
from concourse import mybir
from concourse.bass_utils import run_bass_kernel_spmd

C3_CONNECTIONS = [
    [0, 1, 2], [1, 2, 3], [2, 3, 4], [3, 4, 5], [4, 5, 0], [5, 0, 1],
    [0, 1, 2, 3], [1, 2, 3, 4], [2, 3, 4, 5], [3, 4, 5, 0], [4, 5, 0, 1],
    [5, 0, 1, 2], [0, 1, 3, 4], [1, 2, 4, 5], [0, 2, 3, 5],
    [0, 1, 2, 3, 4, 5],
]

B, CIN, H, W_IMG = 64, 6, 256, 256
COUT, KH, KW = 16, 5, 5
OH = OW = 252
N_CORES = 8
PER = B // N_CORES
NPAIR = PER // 2
G, S, Q = 6, 10, 2            # out rows / in rows / col shifts per band
K, M = S * CIN * Q, G * COUT  # 120, 96
NFREE = 2 * OW                # 504
NB = 42                       # bands per image pair, stride 6 (42*6 = 252)
ND = 3                        # dj-groups
BANDW = 2 * W_IMG             # 512 free elems per (s,c,q) band row

BF = mybir.dt.bfloat16
F32 = mybir.dt.float32
NP_BF = ml_dtypes.bfloat16

SCALE_IN = 2.0 / 3.0
SCALE_OUT = 1.7159


def _mask() -> np.ndarray:
    m = np.zeros((COUT, CIN, KH, KW), dtype=np.float32)
    for i, conn in enumerate(C3_CONNECTIONS):
        m[i, conn] = 1.0
    return m


def _pack_weights(Wm: np.ndarray) -> np.ndarray:
    """[16,6,5,5] -> [K, ND*M].
    row = s*12 + c*2 + q ; col = d*96 + i*16 + o ; value W[o,c,s-i,2d+q]."""
    wp = np.zeros((S, CIN, Q, ND, G, COUT), dtype=np.float32)
    for d in range(ND):
        for q in range(Q):
            dj = 2 * d + q
            if dj >= KW:
                continue
            for i in range(G):
                for di in range(KH):
                    s = i + di
                    wp[s, :, q, d, i, :] = Wm[:, :, di, dj].T
    return wp.reshape(K, ND * M)


def _pack_bias(b: np.ndarray) -> np.ndarray:
    """[16] -> [M, 1]: (2/3)*b[o] at partition i*16+o."""
    bm = np.empty((G, COUT), dtype=np.float32)
    bm[:] = SCALE_IN * b[None, :]
    return bm.reshape(M, 1)


def _pack_x(xs_core: np.ndarray, dtype) -> np.ndarray:
    """[PER,6,256,256] -> [K, NPAIR*NB*512] partition-major q-shifted bands.

    x_packed[s*12+c*2+q, ((n*NB+b)*2+m)*256 + w] = x[n,m,c,6b+s,w+q]
    (column 256 zero-padded)."""
    xs_pad = np.zeros((NPAIR, 2, CIN, H, W_IMG + 1), dtype=np.float32)
    xs_pad[:, :, :, :, :W_IMG] = xs_core.reshape(NPAIR, 2, CIN, H, W_IMG)
    out = np.zeros((S, CIN, Q, NPAIR, NB, 2, W_IMG), dtype=dtype)
    st = xs_pad.strides
    for q in range(Q):
        # bands[n, m, c, b, s, w] = xs_pad[n, m, c, 6b+s, w+q]
        base = xs_pad[:, :, :, :, q:]
        bands = np.lib.stride_tricks.as_strided(
            base,
            shape=(NPAIR, 2, CIN, NB, S, W_IMG),
            strides=(st[0], st[1], st[2], 6 * st[3], st[3], st[4]),
        )
        # -> [s, c, n, b, m, w]
        out[:, :, q] = bands.transpose(4, 2, 0, 3, 1, 5)
    # [s,c,q,n,b,m,w] -> [n, (s,c,q), (b,m,w)]
    return np.ascontiguousarray(out.transpose(3, 0, 1, 2, 4, 5, 6)).reshape(
        NPAIR, K, NB * BANDW)


def _unpack_y_into(y_dev: np.ndarray, out: np.ndarray) -> None:
    """[NPAIR, 96, NB*504] -> out [PER,16,252,252] (unscaled).
    partition p = i*16 + o ; free = b*504 + m*252 + j ; row = 6b + i."""
    yd = y_dev.reshape(NPAIR, G, COUT, NB, 2, OW)
    # -> [pair, m, o, b, i, j]
    yd = yd.transpose(0, 4, 2, 3, 1, 5)
    out.reshape(NPAIR, 2, COUT, NB * G, OW)[...] = yd.reshape(
        NPAIR, 2, COUT, NB * G, OW)


def _split_excess_syncs(nc):
    def budget(ins):
        return 1 if isinstance(ins, (mybir.InstDrain, mybir.InstNoOp)) else 2

    for bb in nc.m.functions[0].blocks:
        new_insts = []
        for ins in bb.instructions:
            si = ins.sync_info
            w = list(si.on_wait) if si and si.on_wait else []
            u = list(si.on_update) if si and si.on_update else []
            cap = budget(ins)
            if len(w) + len(u) > cap:
                keep_n = max(0, cap - len(u))
                excess, kept = w[: len(w) - keep_n], w[len(w) - keep_n:]
                for wait in excess:
                    new_insts.append(
                        mybir.InstNoOp(
                            name=nc.get_next_instruction_name(),
                            sync_info=mybir.SyncInfo(on_wait=[wait],
                                                     on_update=[]),
                            bass_nofuse=True,
                            engine=ins.engine,
                        )
                    )
                ins.sync_info = mybir.SyncInfo(on_wait=kept, on_update=u)
            new_insts.append(ins)
        bb.instructions[:] = new_insts


def _build_nc(iters: int = 1):
    nc = bass.Bass()
    x = nc.declare_dram_parameter("x", [NPAIR, K, NB * BANDW], BF,
                                  isOutput=False)
    wm = nc.declare_dram_parameter("wm", [K, ND * M], BF, isOutput=False)
    bm = nc.declare_dram_parameter("bm", [M, 1], F32, isOutput=False)
    y = nc.declare_dram_parameter("y", [NPAIR, M, NB * NFREE], BF,
                                  isOutput=True)

    with tile.TileContext(nc) as tc:
        with (
            tc.tile_pool(name="consts", bufs=1) as consts,
            tc.tile_pool(name="xp", bufs=2) as xpool,
            tc.tile_pool(name="ps", bufs=8, space="PSUM") as pspool,
            tc.tile_pool(name="op", bufs=2) as opool,
        ):
            wt = consts.tile([K, ND * M], BF)
            nc.sync.dma_start(out=wt[:, :], in_=wm[:, :])
            bt = consts.tile([M, 1], F32)
            nc.sync.dma_start(out=bt[:, :], in_=bm[:, :])
            warm = consts.tile([1, 1], F32)
            nc.scalar.activation(out=warm[:, :], in_=bt[0:1, :],
                                 func=mybir.ActivationFunctionType.Tanh)

            def body(_iv=None):
                for pair in range(NPAIR):
                    xt = xpool.tile([K, NB * BANDW + 8], BF, tag="xt",
                                    name="xt")
                    nc.sync.dma_start(out=xt[:, : NB * BANDW],
                                      in_=x[pair, :, :])
                    og = opool.tile([M, NB * NFREE], BF, tag="og", name="og")
                    for s0 in range(0, NB, 8):
                        nbk = min(8, NB - s0)
                        tc.no_sync_barrier()
                        ps = [
                            pspool.tile([M, NFREE], F32, tag="ps", name="ps")
                            for _ in range(nbk)
                        ]
                        for d in range(ND):
                            c0 = d * M
                            for bb in range(nbk):
                                bidx = s0 + bb
                                xv = xt[:, bidx * BANDW:
                                        (bidx + 1) * BANDW].rearrange(
                                    "k (m w) -> k m w", m=2)
                                nc.tensor.matmul(
                                    ps[bb][:, :],
                                    wt[:, c0:c0 + M],
                                    xv[:, :, 2 * d:2 * d + OW],
                                    start=(d == 0),
                                    stop=(d == ND - 1),
                                )
                        for bb in range(nbk):
                            bo = s0 + bb
                            nc.scalar.activation(
                                out=og[:, bo * NFREE:(bo + 1) * NFREE],
                                in_=ps[bb][:, :],
                                func=mybir.ActivationFunctionType.Tanh,
                                bias=bt[:, 0:1],
                                scale=SCALE_IN,
                            )
                    nc.gpsimd.dma_start(out=y[pair][:, :], in_=og[:, :])

            for _ in range(iters):
                body()
    _split_excess_syncs(nc)
    return nc


_NC_CACHE = {}
LAST_EXEC_NS = None


def kernel(x: np.ndarray, W: np.ndarray, b: np.ndarray) -> np.ndarray:
    global LAST_EXEC_NS
    x = np.asarray(x, dtype=np.float32)
    W = np.asarray(W, dtype=np.float32)
    b = np.asarray(b, dtype=np.float32)

    wp = _pack_weights(W * _mask()).astype(NP_BF)
    bm = _pack_bias(b)
    xs = x.reshape(N_CORES, PER, CIN, H, W_IMG)

    iters = int(os.environ.get("KERNEL_ITERS", "1"))
    if iters not in _NC_CACHE:
        _NC_CACHE[iters] = _build_nc(iters)
    nc = _NC_CACHE[iters]

    in_maps = [
        {"x": _pack_x(xs[i], dtype=NP_BF), "wm": wp, "bm": bm}
        for i in range(N_CORES)
    ]
    res = run_bass_kernel_spmd(nc, in_maps, list(range(N_CORES)))
    LAST_EXEC_NS = res.exec_time_ns
    y = np.empty((B, COUT, OH, OW), dtype=np.float32)
    for i in range(N_CORES):
        _unpack_y_into(
            np.asarray(res.results[i]["y"], dtype=np.float32),
            y[i * PER:(i + 1) * PER],
        )
    y *= np.float32(SCALE_OUT)
    return y



# revision 2
# speedup vs baseline: 1.5097x; 1.5097x over previous
"""nn_C3PartialConv — v7: row-tiled K=60 band pairs, no q-duplication.

Banded Toeplitz formulation with PE row-tiling instead of column-shift
(q) folding:
  - Band = 6 output rows (needs 10 input rows x 6 cin = K 60).
  - Two bands run CONCURRENTLY as row tiles of the 128x128 PE array:
    even band on partitions 0-59 (tile_position (0,0)), odd band on
    partitions 64-123 (tile_position (64,0)).  One XBUS column carries
    both streams on disjoint partition lanes.
  - The 5 kernel columns dj are 5 accumulating matmuls per band with a
    free-dim window shift (252 + 4 = 256, so no pad column needed).
  - M = 96 = 6 out rows x 16 cout; N = 504 = 2 images x 252 cols.
  - x HBM traffic halves vs the q-folded v6 (no duplicated shift copy);
    x loads are chunked (3 x ~0.9 MB per pair) so the first matmul
    waits only on the first chunk.
  - tanh+bias applied by ScalarE over FOUR PSUM banks per instruction
    ([96, 2040], bands padded to 512 f32 = one bank each) to amortize
    the ~352-cycle ACT overhead; the 8 garbage columns between bands
    are dropped on the host.
  - og stores are chunked per x-chunk (1.37 MB) to overlap output DMA
    and shrink the tail.
"""

import os
import numpy as np
import ml_dtypes

import concourse.bass as bass
import concourse.tile as tile
from concourse import mybir
from concourse.bass_utils import run_bass_kernel_spmd

C3_CONNECTIONS = [
    [0, 1, 2], [1, 2, 3], [2, 3, 4], [3, 4, 5], [4, 5, 0], [5, 0, 1],
    [0, 1, 2, 3], [1, 2, 3, 4], [2, 3, 4, 5], [3, 4, 5, 0], [4, 5, 0, 1],
    [5, 0, 1, 2], [0, 1, 3, 4], [1, 2, 4, 5], [0, 2, 3, 5],
    [0, 1, 2, 3, 4, 5],
]

B, CIN, H, W_IMG = 64, 6, 256, 256
COUT, KH, KW = 16, 5, 5
OH = OW = 252
N_CORES = 8
PER = B // N_CORES          # 8 images per core
NPAIR = PER // 2            # 4 image pairs per core
G, S = 6, 10                # out rows / in rows per band
K, M = S * CIN, G * COUT    # 60, 96
NFREE = 2 * OW              # 504
NB = 42                     # bands per image pair (42*6 = 252)
NBP = NB // 2               # 21 band pairs
BANDW = 2 * W_IMG           # 512 free elems per (s,c) band row
CHUNK_BP = 7                # band pairs per x chunk
NCHUNK = NBP // CHUNK_BP    # 3
XCH = CHUNK_BP * BANDW      # 3584 x cols per chunk
# psum blocks per chunk: (local bp start, n bp); [96,2048] f32 = 4 banks
BLOCKS = [(0, 2), (2, 2), (4, 2), (6, 1)]
BLK_COLS = [0, 2040, 4080, 6120]      # og col offset per block
OG_CH = 3 * 2040 + 1016               # 7136 og cols per chunk

BF = mybir.dt.bfloat16
F32 = mybir.dt.float32
NP_BF = ml_dtypes.bfloat16

SCALE_IN = 2.0 / 3.0
SCALE_OUT = 1.7159


def _mask() -> np.ndarray:
    m = np.zeros((COUT, CIN, KH, KW), dtype=np.float32)
    for i, conn in enumerate(C3_CONNECTIONS):
        m[i, conn] = 1.0
    return m


def _pack_weights(Wm: np.ndarray) -> np.ndarray:
    """[16,6,5,5] -> [128, 5*96].
    row p = s*6 + c (and 64+p duplicate); col = dj*96 + i*16 + o;
    value Wm[o, c, s-i, dj] for 0 <= s-i < 5."""
    wp = np.zeros((64, KW, G, COUT), dtype=np.float32)
    for i in range(G):
        for di in range(KH):
            s = i + di
            for c in range(CIN):
                wp[s * CIN + c, :, i, :] = Wm[:, c, di, :].T
    out = np.zeros((128, KW * M), dtype=np.float32)
    out[0:64] = wp.reshape(64, KW * M)
    out[64:128] = out[0:64]
    return out


def _pack_bias(b: np.ndarray) -> np.ndarray:
    """[16] -> [M, 1]: (2/3)*b[o] at partition i*16+o."""
    bm = np.empty((G, COUT), dtype=np.float32)
    bm[:] = SCALE_IN * b[None, :]
    return bm.reshape(M, 1)


def _pack_x(xs_core: np.ndarray, dtype) -> np.ndarray:
    """[PER,6,256,256] -> [NPAIR, 128, NBP*512].

    Partition p = s*6+c holds even-band rows (image row 12j+s), partition
    64+p holds odd-band rows (image row 12j+6+s); free = j*512 + m*256 + w.
    """
    xp = xs_core.reshape(NPAIR, 2, CIN, H, W_IMG)
    st = xp.strides
    # full[n, m, c, j, t, w] = xp[n, m, c, 12j + t, w],  t in 0..15
    full = np.lib.stride_tricks.as_strided(
        xp,
        shape=(NPAIR, 2, CIN, NBP, 16, W_IMG),
        strides=(st[0], st[1], st[2], 12 * st[3], st[3], st[4]),
    )
    out = np.zeros((NPAIR, 128, NBP, 2, W_IMG), dtype=dtype)
    # -> [n, s, c, j, m, w]
    out[:, 0:K] = full[:, :, :, :, 0:S].transpose(0, 4, 2, 3, 1, 5).reshape(
        NPAIR, K, NBP, 2, W_IMG)
    out[:, 64:64 + K] = full[:, :, :, :, 6:6 + S].transpose(
        0, 4, 2, 3, 1, 5).reshape(NPAIR, K, NBP, 2, W_IMG)
    return out.reshape(NPAIR, 128, NBP * BANDW)


def _unpack_y_into(y_dev: np.ndarray, out: np.ndarray) -> None:
    """[NPAIR, NCHUNK, 96, 7136] -> out [PER,16,252,252] (unscaled).

    Per chunk: 3 blocks of 2040 cols (4 bands at 512-col stride, 504
    valid) + 1 block of 1016 cols (2 bands).  Band b=2j+h covers out
    rows 6b+i; partition p = i*16+o; in-band col = m*252 + wc.
    """
    bands = np.empty((NPAIR, M, NB, NFREE), dtype=y_dev.dtype)
    for ch in range(NCHUNK):
        blk = y_dev[:, ch, :, : 3 * 2040].reshape(NPAIR, M, 3, 2040)
        st = blk.strides
        four = np.lib.stride_tricks.as_strided(
            blk, shape=(NPAIR, M, 3, 4, NFREE),
            strides=(st[0], st[1], st[2], 512 * st[3], st[3]),
        )
        b0 = ch * (2 * CHUNK_BP)
        bands[:, :, b0:b0 + 12] = four.reshape(NPAIR, M, 12, NFREE)
        rest = y_dev[:, ch, :, 3 * 2040:]
        bands[:, :, b0 + 12] = rest[:, :, 0:NFREE]
        bands[:, :, b0 + 13] = rest[:, :, 512:512 + NFREE]
    # [n, i, o, b, m, w] -> [n, m, o, (b, i), w]
    yd = bands.reshape(NPAIR, G, COUT, NB, 2, OW).transpose(0, 4, 2, 3, 1, 5)
    out.reshape(NPAIR, 2, COUT, NB * G, OW)[...] = yd.reshape(
        NPAIR, 2, COUT, NB * G, OW)


def _split_excess_syncs(nc):
    def budget(ins):
        return 1 if isinstance(ins, (mybir.InstDrain, mybir.InstNoOp)) else 2

    for bb in nc.m.functions[0].blocks:
        new_insts = []
        for ins in bb.instructions:
            si = ins.sync_info
            w = list(si.on_wait) if si and si.on_wait else []
            u = list(si.on_update) if si and si.on_update else []
            cap = budget(ins)
            if len(w) + len(u) > cap:
                keep_n = max(0, cap - len(u))
                excess, kept = w[: len(w) - keep_n], w[len(w) - keep_n:]
                for wait in excess:
                    new_insts.append(
                        mybir.InstNoOp(
                            name=nc.get_next_instruction_name(),
                            sync_info=mybir.SyncInfo(on_wait=[wait],
                                                     on_update=[]),
                            bass_nofuse=True,
                            engine=ins.engine,
                        )
                    )
                ins.sync_info = mybir.SyncInfo(on_wait=kept, on_update=u)
            new_insts.append(ins)
        bb.instructions[:] = new_insts


def _build_nc(iters: int = 1):
    nc = bass.Bass()
    x = nc.declare_dram_parameter("x", [NPAIR, 128, NBP * BANDW], BF,
                                  isOutput=False)
    wm = nc.declare_dram_parameter("wm", [128, KW * M], BF, isOutput=False)
    bm = nc.declare_dram_parameter("bm", [M, 1], F32, isOutput=False)
    y = nc.declare_dram_parameter("y", [NPAIR, NCHUNK, M, OG_CH], BF,
                                  isOutput=True)

    with tile.TileContext(nc) as tc:
        with (
            tc.tile_pool(name="consts", bufs=1) as consts,
            tc.tile_pool(name="xp", bufs=3) as xpool,
            tc.tile_pool(name="ps", bufs=2, space="PSUM") as pspool,
            tc.tile_pool(name="op", bufs=2) as opool,
        ):
            wt = consts.tile([128, KW * M], BF)
            nc.sync.dma_start(out=wt[:, :], in_=wm[:, :])
            bt = consts.tile([M, 1], F32)
            nc.sync.dma_start(out=bt[:, :], in_=bm[:, :])
            # preload the tanh table set
            warm = consts.tile([1, 1], F32)
            nc.scalar.activation(out=warm[:, :], in_=bt[0:1, :],
                                 func=mybir.ActivationFunctionType.Tanh)
            # HAM warm-up: ~4us of tiny matmuls during the first x load
            wps = pspool.tile([M, 2048], F32, tag="ps", name="wps")
            for _ in range(64):
                nc.tensor.matmul(wps[0:16, 0:128], wt[0:K, 0:16],
                                 wt[0:K, 0:128], start=True, stop=True)

            def body(_iv=None):
                for pair in range(NPAIR):
                    for ch in range(NCHUNK):
                        xt = xpool.tile([128, XCH], BF, tag="xt", name="xt")
                        nc.sync.dma_start(
                            out=xt[:, :],
                            in_=x[pair, :, ch * XCH:(ch + 1) * XCH])
                        og = opool.tile([M, OG_CH], BF, tag="og", name="og")
                        for (lb0, bsz), goff in zip(BLOCKS, BLK_COLS):
                            tc.no_sync_barrier()
                            ps = pspool.tile([M, 2048], F32, tag="ps",
                                             name="ps")
                            for dj in range(KW):
                                for u in range(bsz):
                                    lv = lb0 + u
                                    xv = xt[:, lv * BANDW:
                                            (lv + 1) * BANDW].rearrange(
                                        "k (m w) -> k m w", m=2)
                                    for h in range(2):
                                        off = u * 1024 + h * 512
                                        nc.tensor.matmul(
                                            ps[:, off:off + NFREE],
                                            wt[h * 64:h * 64 + K,
                                               dj * M:(dj + 1) * M],
                                            xv[h * 64:h * 64 + K, :,
                                               dj:dj + OW],
                                            start=(dj == 0),
                                            stop=(dj == KW - 1),
                                        )
                            width = 2040 if bsz == 2 else 1016
                            nc.scalar.activation(
                                out=og[:, goff:goff + width],
                                in_=ps[:, 0:width],
                                func=mybir.ActivationFunctionType.Tanh,
                                bias=bt[:, 0:1],
                                scale=SCALE_IN,
                            )
                        nc.gpsimd.dma_start(out=y[pair, ch][:, :],
                                            in_=og[:, :])

            for _ in range(iters):
                body()
    _split_excess_syncs(nc)
    return nc


_NC_CACHE = {}
LAST_EXEC_NS = None


def kernel(x: np.ndarray, W: np.ndarray, b: np.ndarray) -> np.ndarray:
    global LAST_EXEC_NS
    x = np.asarray(x, dtype=np.float32)
    W = np.asarray(W, dtype=np.float32)
    b = np.asarray(b, dtype=np.float32)

    wp = _pack_weights(W * _mask()).astype(NP_BF)
    bm = _pack_bias(b)
    xs = x.reshape(N_CORES, PER, CIN, H, W_IMG)

    iters = int(os.environ.get("KERNEL_ITERS", "1"))
    if iters not in _NC_CACHE:
        _NC_CACHE[iters] = _build_nc(iters)
    nc = _NC_CACHE[iters]

    in_maps = [
        {"x": _pack_x(xs[i], dtype=NP_BF), "wm": wp, "bm": bm}
        for i in range(N_CORES)
    ]
    res = run_bass_kernel_spmd(nc, in_maps, list(range(N_CORES)))
    LAST_EXEC_NS = res.exec_time_ns
    y = np.empty((B, COUT, OH, OW), dtype=np.float32)
    for i in range(N_CORES):
        _unpack_y_into(
            np.asarray(res.results[i]["y"], dtype=np.float32),
            y[i * PER:(i + 1) * PER],
        )
    y *= np.float32(SCALE_OUT)
    return y


# revision 3
# speedup vs baseline: 1.6276x; 1.0780x over previous
"""nn_C3PartialConv — v7.1: row-tiled K=60 band pairs, no q-duplication.

Banded Toeplitz formulation with PE row-tiling instead of column-shift
(q) folding:
  - Band = 6 output rows (needs 10 input rows x 6 cin = K 60).
  - Two bands run CONCURRENTLY as row tiles of the 128x128 PE array:
    even band on partitions 0-59 (tile_position (0,0)), odd band on
    partitions 64-123 (tile_position (64,0)).  One XBUS column carries
    both streams on disjoint partition lanes.
  - The 5 kernel columns dj are 5 accumulating matmuls per band with a
    free-dim window shift (252 + 4 = 256, so no pad column needed).
  - M = 96 = 6 out rows x 16 cout; N = 504 = 2 images x 252 cols.
  - x HBM traffic halves vs the q-folded v6 (no duplicated shift copy);
    x loads are chunked (5/8/8 band pairs) so the first matmul waits
    only on the first chunk, and only ONE odd 1-band-pair PSUM block
    per pair (block shorter than the ACT it waits on => PE stall).
  - tanh+bias applied by ScalarE over FOUR PSUM banks per instruction
    ([96, 2040], bands padded to 512 f32 = one bank each) to amortize
    the ~352-cycle ACT overhead; the 8 garbage columns between bands
    are dropped on the host.
  - og stores are split in two per chunk to overlap output DMA and
    shrink the tail.
"""

import os
import numpy as np
import ml_dtypes

import concourse.bass as bass
import concourse.tile as tile
from concourse import mybir
from concourse.bass_utils import run_bass_kernel_spmd

C3_CONNECTIONS = [
    [0, 1, 2], [1, 2, 3], [2, 3, 4], [3, 4, 5], [4, 5, 0], [5, 0, 1],
    [0, 1, 2, 3], [1, 2, 3, 4], [2, 3, 4, 5], [3, 4, 5, 0], [4, 5, 0, 1],
    [5, 0, 1, 2], [0, 1, 3, 4], [1, 2, 4, 5], [0, 2, 3, 5],
    [0, 1, 2, 3, 4, 5],
]

B, CIN, H, W_IMG = 64, 6, 256, 256
COUT, KH, KW = 16, 5, 5
OH = OW = 252
N_CORES = 8
PER = B // N_CORES          # 8 images per core
NPAIR = PER // 2            # 4 image pairs per core
G, S = 6, 10                # out rows / in rows per band
K, M = S * CIN, G * COUT    # 60, 96
NFREE = 2 * OW              # 504
NB = 42                     # bands per image pair (42*6 = 252)
NBP = NB // 2               # 21 band pairs
BANDW = 2 * W_IMG           # 512 free elems per (s,c) band row

# x chunks per pair: (bp start, n bp); first is small so the pipeline
# fills fast, and only chunk 0 carries the odd 1-bp PSUM block.
CHUNKS = [(0, 5), (5, 8), (13, 8)]
# psum blocks per chunk size: (local bp start, n bp)
BLOCKS_5 = [(0, 2), (2, 2), (4, 1)]
BLOCKS_8 = [(0, 2), (2, 2), (4, 2), (6, 2)]
OG_5 = 2 * 2040 + 1016      # 5096
OG_8 = 4 * 2040             # 8160
OG_PAIR = OG_5 + 2 * OG_8   # 21416 og cols per pair

BF = mybir.dt.bfloat16
F32 = mybir.dt.float32
NP_BF = ml_dtypes.bfloat16

SCALE_IN = 2.0 / 3.0
SCALE_OUT = 1.7159


def _mask() -> np.ndarray:
    m = np.zeros((COUT, CIN, KH, KW), dtype=np.float32)
    for i, conn in enumerate(C3_CONNECTIONS):
        m[i, conn] = 1.0
    return m


def _pack_weights(Wm: np.ndarray) -> np.ndarray:
    """[16,6,5,5] -> [128, 5*96].
    row p = s*6 + c (and 64+p duplicate); col = dj*96 + i*16 + o;
    value Wm[o, c, s-i, dj] for 0 <= s-i < 5."""
    wp = np.zeros((64, KW, G, COUT), dtype=np.float32)
    for i in range(G):
        for di in range(KH):
            s = i + di
            for c in range(CIN):
                wp[s * CIN + c, :, i, :] = Wm[:, c, di, :].T
    out = np.zeros((128, KW * M), dtype=np.float32)
    out[0:64] = wp.reshape(64, KW * M)
    out[64:128] = out[0:64]
    return out


def _pack_bias(b: np.ndarray) -> np.ndarray:
    """[16] -> [M, 1]: (2/3)*b[o] at partition i*16+o."""
    bm = np.empty((G, COUT), dtype=np.float32)
    bm[:] = SCALE_IN * b[None, :]
    return bm.reshape(M, 1)


def _pack_x(xs_core: np.ndarray, dtype) -> np.ndarray:
    """[PER,6,256,256] -> [NPAIR, 128, NBP*512].

    Partition p = s*6+c holds even-band rows (image row 12j+s), partition
    64+p holds odd-band rows (image row 12j+6+s); free = j*512 + m*256 + w.
    """
    xp = xs_core.reshape(NPAIR, 2, CIN, H, W_IMG)
    st = xp.strides
    # full[n, m, c, j, t, w] = xp[n, m, c, 12j + t, w],  t in 0..15
    full = np.lib.stride_tricks.as_strided(
        xp,
        shape=(NPAIR, 2, CIN, NBP, 16, W_IMG),
        strides=(st[0], st[1], st[2], 12 * st[3], st[3], st[4]),
    )
    out = np.zeros((NPAIR, 128, NBP, 2, W_IMG), dtype=dtype)
    # -> [n, s, c, j, m, w]
    out[:, 0:K] = full[:, :, :, :, 0:S].transpose(0, 4, 2, 3, 1, 5).reshape(
        NPAIR, K, NBP, 2, W_IMG)
    out[:, 64:64 + K] = full[:, :, :, :, 6:6 + S].transpose(
        0, 4, 2, 3, 1, 5).reshape(NPAIR, K, NBP, 2, W_IMG)
    return out.reshape(NPAIR, 128, NBP * BANDW)


# og column segments per pair: (col, n bands); bands are 504 valid cols
# at 512-col stride within a segment; segments cover bands in order.
def _og_segments():
    segs = []
    col = 0
    for _, nbp in CHUNKS:
        blocks = BLOCKS_5 if nbp == 5 else BLOCKS_8
        for _, bsz in blocks:
            segs.append((col, 2 * bsz))
            col += 2040 if bsz == 2 else 1016
    assert col == OG_PAIR
    return segs


OG_SEGS = _og_segments()


def _unpack_y_into(y_dev: np.ndarray, out: np.ndarray) -> None:
    """[NPAIR, 96, OG_PAIR] -> out [PER,16,252,252] (unscaled)."""
    bands = np.empty((NPAIR, M, NB, NFREE), dtype=y_dev.dtype)
    b = 0
    st = y_dev.strides
    for col, nb in OG_SEGS:
        seg = np.lib.stride_tricks.as_strided(
            y_dev[:, :, col:],
            shape=(NPAIR, M, nb, NFREE),
            strides=(st[0], st[1], 512 * st[2], st[2]),
        )
        bands[:, :, b:b + nb] = seg
        b += nb
    # [n, i, o, b, m, w] -> [n, m, o, (b, i), w]
    yd = bands.reshape(NPAIR, G, COUT, NB, 2, OW).transpose(0, 4, 2, 3, 1, 5)
    out.reshape(NPAIR, 2, COUT, NB * G, OW)[...] = yd.reshape(
        NPAIR, 2, COUT, NB * G, OW)


def _split_excess_syncs(nc):
    def budget(ins):
        return 1 if isinstance(ins, (mybir.InstDrain, mybir.InstNoOp)) else 2

    for bb in nc.m.functions[0].blocks:
        new_insts = []
        for ins in bb.instructions:
            si = ins.sync_info
            w = list(si.on_wait) if si and si.on_wait else []
            u = list(si.on_update) if si and si.on_update else []
            cap = budget(ins)
            if len(w) + len(u) > cap:
                keep_n = max(0, cap - len(u))
                excess, kept = w[: len(w) - keep_n], w[len(w) - keep_n:]
                for wait in excess:
                    new_insts.append(
                        mybir.InstNoOp(
                            name=nc.get_next_instruction_name(),
                            sync_info=mybir.SyncInfo(on_wait=[wait],
                                                     on_update=[]),
                            bass_nofuse=True,
                            engine=ins.engine,
                        )
                    )
                ins.sync_info = mybir.SyncInfo(on_wait=kept, on_update=u)
            new_insts.append(ins)
        bb.instructions[:] = new_insts


def _build_nc(iters: int = 1):
    nc = bass.Bass()
    x = nc.declare_dram_parameter("x", [NPAIR, 128, NBP * BANDW], BF,
                                  isOutput=False)
    wm = nc.declare_dram_parameter("wm", [128, KW * M], BF, isOutput=False)
    bm = nc.declare_dram_parameter("bm", [M, 1], F32, isOutput=False)
    y = nc.declare_dram_parameter("y", [NPAIR, M, OG_PAIR], BF,
                                  isOutput=True)

    with tile.TileContext(nc) as tc:
        with (
            tc.tile_pool(name="consts", bufs=1) as consts,
            tc.tile_pool(name="x5", bufs=2) as x5pool,
            tc.tile_pool(name="x8", bufs=3) as x8pool,
            tc.tile_pool(name="ps", bufs=2, space="PSUM") as pspool,
            tc.tile_pool(name="o5", bufs=2) as o5pool,
            tc.tile_pool(name="o8", bufs=2) as o8pool,
        ):
            wt = consts.tile([128, KW * M], BF)
            nc.sync.dma_start(out=wt[:, :], in_=wm[:, :])
            bt = consts.tile([M, 1], F32)
            nc.sync.dma_start(out=bt[:, :], in_=bm[:, :])
            # preload the tanh table set
            warm = consts.tile([1, 1], F32)
            nc.scalar.activation(out=warm[:, :], in_=bt[0:1, :],
                                 func=mybir.ActivationFunctionType.Tanh)
            # HAM warm-up: ~4us of tiny matmuls during the first x load
            wps = pspool.tile([M, 2048], F32, tag="ps", name="wps")
            for _ in range(64):
                nc.tensor.matmul(wps[0:16, 0:128], wt[0:K, 0:16],
                                 wt[0:K, 0:128], start=True, stop=True)

            def body(_iv=None):
                for pair in range(NPAIR):
                    ogcol = 0
                    for bp0, nbp in CHUNKS:
                        small = nbp == 5
                        xt = (x5pool if small else x8pool).tile(
                            [128, nbp * BANDW], BF,
                            tag="x5" if small else "x8", name="xt")
                        nc.sync.dma_start(
                            out=xt[:, :],
                            in_=x[pair, :,
                                  bp0 * BANDW:(bp0 + nbp) * BANDW])
                        ogw = OG_5 if small else OG_8
                        og = (o5pool if small else o8pool).tile(
                            [M, ogw], BF,
                            tag="o5" if small else "o8", name="og")
                        goff = 0
                        for lb0, bsz in (BLOCKS_5 if small else BLOCKS_8):
                            tc.no_sync_barrier()
                            ps = pspool.tile([M, 2048], F32, tag="ps",
                                             name="ps")
                            for dj in range(KW):
                                for u in range(bsz):
                                    lv = lb0 + u
                                    xv = xt[:, lv * BANDW:
                                            (lv + 1) * BANDW].rearrange(
                                        "k (m w) -> k m w", m=2)
                                    for h in range(2):
                                        off = u * 1024 + h * 512
                                        nc.tensor.matmul(
                                            ps[:, off:off + NFREE],
                                            wt[h * 64:h * 64 + K,
                                               dj * M:(dj + 1) * M],
                                            xv[h * 64:h * 64 + K, :,
                                               dj:dj + OW],
                                            start=(dj == 0),
                                            stop=(dj == KW - 1),
                                        )
                            width = 2040 if bsz == 2 else 1016
                            nc.scalar.activation(
                                out=og[:, goff:goff + width],
                                in_=ps[:, 0:width],
                                func=mybir.ActivationFunctionType.Tanh,
                                bias=bt[:, 0:1],
                                scale=SCALE_IN,
                            )
                            goff += width
                        # split store: overlap output DMA, shrink tail
                        half = 4080
                        nc.gpsimd.dma_start(
                            out=y[pair][:, ogcol:ogcol + half],
                            in_=og[:, 0:half])
                        nc.gpsimd.dma_start(
                            out=y[pair][:, ogcol + half:ogcol + ogw],
                            in_=og[:, half:ogw])
                        ogcol += ogw

            for _ in range(iters):
                body()
    _split_excess_syncs(nc)
    return nc


_NC_CACHE = {}
LAST_EXEC_NS = None


def kernel(x: np.ndarray, W: np.ndarray, b: np.ndarray) -> np.ndarray:
    global LAST_EXEC_NS
    x = np.asarray(x, dtype=np.float32)
    W = np.asarray(W, dtype=np.float32)
    b = np.asarray(b, dtype=np.float32)

    wp = _pack_weights(W * _mask()).astype(NP_BF)
    bm = _pack_bias(b)
    xs = x.reshape(N_CORES, PER, CIN, H, W_IMG)

    iters = int(os.environ.get("KERNEL_ITERS", "1"))
    if iters not in _NC_CACHE:
        _NC_CACHE[iters] = _build_nc(iters)
    nc = _NC_CACHE[iters]

    in_maps = [
        {"x": _pack_x(xs[i], dtype=NP_BF), "wm": wp, "bm": bm}
        for i in range(N_CORES)
    ]
    res = run_bass_kernel_spmd(nc, in_maps, list(range(N_CORES)))
    LAST_EXEC_NS = res.exec_time_ns
    y = np.empty((B, COUT, OH, OW), dtype=np.float32)
    for i in range(N_CORES):
        _unpack_y_into(
            np.asarray(res.results[i]["y"], dtype=np.float32),
            y[i * PER:(i + 1) * PER],
        )
    y *= np.float32(SCALE_OUT)
    return y


# revision 10
# speedup vs baseline: 1.6720x; 1.0273x over previous
"""nn_C3PartialConv — v7.1: row-tiled K=60 band pairs, no q-duplication.

Banded Toeplitz formulation with PE row-tiling instead of column-shift
(q) folding:
  - Band = 6 output rows (needs 10 input rows x 6 cin = K 60).
  - Two bands run CONCURRENTLY as row tiles of the 128x128 PE array:
    even band on partitions 0-59 (tile_position (0,0)), odd band on
    partitions 64-123 (tile_position (64,0)).  One XBUS column carries
    both streams on disjoint partition lanes.
  - The 5 kernel columns dj are 5 accumulating matmuls per band with a
    free-dim window shift (252 + 4 = 256, so no pad column needed).
  - M = 96 = 6 out rows x 16 cout; N = 504 = 2 images x 252 cols.
  - x HBM traffic halves vs the q-folded v6 (no duplicated shift copy);
    x loads are chunked (5/8/8 band pairs) so the first matmul waits
    only on the first chunk, and only ONE odd 1-band-pair PSUM block
    per pair (block shorter than the ACT it waits on => PE stall).
  - tanh+bias applied by ScalarE over FOUR PSUM banks per instruction
    ([96, 2040], bands padded to 512 f32 = one bank each) to amortize
    the ~352-cycle ACT overhead; the 8 garbage columns between bands
    are dropped on the host.
  - og stores are split in two per chunk to overlap output DMA and
    shrink the tail.
"""

import os
import numpy as np
import ml_dtypes

import concourse.bass as bass
import concourse.tile as tile
from concourse import mybir
from concourse.bass_utils import run_bass_kernel_spmd

C3_CONNECTIONS = [
    [0, 1, 2], [1, 2, 3], [2, 3, 4], [3, 4, 5], [4, 5, 0], [5, 0, 1],
    [0, 1, 2, 3], [1, 2, 3, 4], [2, 3, 4, 5], [3, 4, 5, 0], [4, 5, 0, 1],
    [5, 0, 1, 2], [0, 1, 3, 4], [1, 2, 4, 5], [0, 2, 3, 5],
    [0, 1, 2, 3, 4, 5],
]

B, CIN, H, W_IMG = 64, 6, 256, 256
COUT, KH, KW = 16, 5, 5
OH = OW = 252
N_CORES = 8
PER = B // N_CORES          # 8 images per core
NPAIR = PER // 2            # 4 image pairs per core
G, S = 6, 10                # out rows / in rows per band
K, M = S * CIN, G * COUT    # 60, 96
NFREE = 2 * OW              # 504
NB = 42                     # bands per image pair (42*6 = 252)
NBP = NB // 2               # 21 band pairs
BANDW = 2 * W_IMG           # 512 free elems per (s,c) band row

# x chunks per pair: (bp start, n bp).  Pairs 0-2 lead with the small
# chunk so the pipeline fills fast; the last pair ends with it so the
# final og store (and final ACT) are small => short tail.
CHUNKS_HEAD = [(0, 5), (5, 8), (13, 8)]
CHUNKS_TAIL = [(0, 8), (8, 8), (16, 5)]
# psum blocks per chunk size: (local bp start, n bp)
BLOCKS_5 = [(0, 2), (2, 2), (4, 1)]
BLOCKS_8 = [(0, 2), (2, 2), (4, 2), (6, 2)]
OG_5 = 2 * 2040 + 1016      # 5096
OG_8 = 4 * 2040             # 8160
OG_PAIR = OG_5 + 2 * OG_8   # 21416 og cols per pair

BF = mybir.dt.bfloat16
F32 = mybir.dt.float32
NP_BF = ml_dtypes.bfloat16

SCALE_IN = 2.0 / 3.0
SCALE_OUT = 1.7159


def _mask() -> np.ndarray:
    m = np.zeros((COUT, CIN, KH, KW), dtype=np.float32)
    for i, conn in enumerate(C3_CONNECTIONS):
        m[i, conn] = 1.0
    return m


def _pack_weights(Wm: np.ndarray) -> np.ndarray:
    """[16,6,5,5] -> [128, 5*96].
    row p = s*6 + c (and 64+p duplicate); col = dj*96 + i*16 + o;
    value Wm[o, c, s-i, dj] for 0 <= s-i < 5."""
    wp = np.zeros((64, KW, G, COUT), dtype=np.float32)
    for i in range(G):
        for di in range(KH):
            s = i + di
            for c in range(CIN):
                wp[s * CIN + c, :, i, :] = Wm[:, c, di, :].T
    out = np.zeros((128, KW * M), dtype=np.float32)
    out[0:64] = wp.reshape(64, KW * M)
    out[64:128] = out[0:64]
    return out


def _pack_bias(b: np.ndarray) -> np.ndarray:
    """[16] -> [M, 1]: (2/3)*b[o] at partition i*16+o."""
    bm = np.empty((G, COUT), dtype=np.float32)
    bm[:] = SCALE_IN * b[None, :]
    return bm.reshape(M, 1)


def _pack_x(xs_core: np.ndarray, dtype) -> np.ndarray:
    """[PER,6,256,256] -> [NPAIR, 120, NBP*512].

    Row p = s*6+c holds even-band rows (image row 12j+s), row 60+p holds
    odd-band rows (image row 12j+6+s); free = j*512 + m*256 + w.  On
    device the halves land on SBUF partitions 0-59 / 64-123.
    """
    xp = xs_core.reshape(NPAIR, 2, CIN, H, W_IMG)
    st = xp.strides
    # full[n, m, c, j, t, w] = xp[n, m, c, 12j + t, w],  t in 0..15
    full = np.lib.stride_tricks.as_strided(
        xp,
        shape=(NPAIR, 2, CIN, NBP, 16, W_IMG),
        strides=(st[0], st[1], st[2], 12 * st[3], st[3], st[4]),
    )
    out = np.empty((NPAIR, 2 * K, NBP, 2, W_IMG), dtype=dtype)
    # -> [n, s, c, j, m, w]
    out[:, 0:K] = full[:, :, :, :, 0:S].transpose(0, 4, 2, 3, 1, 5).reshape(
        NPAIR, K, NBP, 2, W_IMG)
    out[:, K:2 * K] = full[:, :, :, :, 6:6 + S].transpose(
        0, 4, 2, 3, 1, 5).reshape(NPAIR, K, NBP, 2, W_IMG)
    return out.reshape(NPAIR, 2 * K, NBP * BANDW)


# og column segments per pair: (col, n bands); bands are 504 valid cols
# at 512-col stride within a segment; segments cover bands in order.
def _og_segments(chunks):
    segs = []
    col = 0
    for _, nbp in chunks:
        blocks = BLOCKS_5 if nbp == 5 else BLOCKS_8
        for _, bsz in blocks:
            segs.append((col, 2 * bsz))
            col += 2040 if bsz == 2 else 1016
    assert col == OG_PAIR
    return segs


OG_SEGS_HEAD = _og_segments(CHUNKS_HEAD)
OG_SEGS_TAIL = _og_segments(CHUNKS_TAIL)


def _unpack_y_into(y_dev: np.ndarray, out: np.ndarray) -> None:
    """[NPAIR, 96, OG_PAIR] -> out [PER,16,252,252] (unscaled)."""
    bands = np.empty((NPAIR, M, NB, NFREE), dtype=y_dev.dtype)
    st = y_dev.strides
    for n in range(NPAIR):
        segs = OG_SEGS_TAIL if n == NPAIR - 1 else OG_SEGS_HEAD
        b = 0
        for col, nb in segs:
            seg = np.lib.stride_tricks.as_strided(
                y_dev[n, :, col:],
                shape=(M, nb, NFREE),
                strides=(st[1], 512 * st[2], st[2]),
            )
            bands[n, :, b:b + nb] = seg
            b += nb
    # [n, i, o, b, m, w] -> [n, m, o, (b, i), w]
    yd = bands.reshape(NPAIR, G, COUT, NB, 2, OW).transpose(0, 4, 2, 3, 1, 5)
    out.reshape(NPAIR, 2, COUT, NB * G, OW)[...] = yd.reshape(
        NPAIR, 2, COUT, NB * G, OW)


def _split_excess_syncs(nc):
    def budget(ins):
        return 1 if isinstance(ins, (mybir.InstDrain, mybir.InstNoOp)) else 2

    for bb in nc.m.functions[0].blocks:
        new_insts = []
        for ins in bb.instructions:
            si = ins.sync_info
            w = list(si.on_wait) if si and si.on_wait else []
            u = list(si.on_update) if si and si.on_update else []
            cap = budget(ins)
            if len(w) + len(u) > cap:
                keep_n = max(0, cap - len(u))
                excess, kept = w[: len(w) - keep_n], w[len(w) - keep_n:]
                for wait in excess:
                    new_insts.append(
                        mybir.InstNoOp(
                            name=nc.get_next_instruction_name(),
                            sync_info=mybir.SyncInfo(on_wait=[wait],
                                                     on_update=[]),
                            bass_nofuse=True,
                            engine=ins.engine,
                        )
                    )
                ins.sync_info = mybir.SyncInfo(on_wait=kept, on_update=u)
            new_insts.append(ins)
        bb.instructions[:] = new_insts


def _build_nc(iters: int = 1):
    nc = bass.Bass()
    x = nc.declare_dram_parameter("x", [NPAIR, 2 * K, NBP * BANDW], BF,
                                  isOutput=False)
    wm = nc.declare_dram_parameter("wm", [128, KW * M], BF, isOutput=False)
    bm = nc.declare_dram_parameter("bm", [M, 1], F32, isOutput=False)
    y = nc.declare_dram_parameter("y", [NPAIR, M, OG_PAIR], BF,
                                  isOutput=True)

    with tile.TileContext(nc) as tc:
        with (
            tc.tile_pool(name="consts", bufs=1) as consts,
            tc.tile_pool(name="x5", bufs=2) as x5pool,
            tc.tile_pool(name="x8", bufs=3) as x8pool,
            tc.tile_pool(name="ps", bufs=2, space="PSUM") as pspool,
            tc.tile_pool(name="o5", bufs=2) as o5pool,
            tc.tile_pool(name="o8", bufs=2) as o8pool,
        ):
            wt = consts.tile([128, KW * M], BF)
            nc.sync.dma_start(out=wt[:, :], in_=wm[:, :])
            bt = consts.tile([M, 1], F32)
            nc.sync.dma_start(out=bt[:, :], in_=bm[:, :])
            # preload the tanh table set
            warm = consts.tile([1, 1], F32)
            nc.scalar.activation(out=warm[:, :], in_=bt[0:1, :],
                                 func=mybir.ActivationFunctionType.Tanh)
            # HAM warm-up: ~4us of tiny matmuls during DMA-queue spin-up;
            # memset-sourced so they need no DMA at all.
            wmt = consts.tile([K, 128], BF)
            nc.vector.memset(wmt[:, :], 0.25)
            wps = pspool.tile([M, 2048], F32, tag="ps", name="wps")
            for _ in range(48):
                nc.tensor.matmul(wps[0:16, 0:128], wmt[:, 0:16],
                                 wmt[:, :], start=True, stop=True)

            def body(_iv=None):
                for pair in range(NPAIR):
                    chunks = CHUNKS_TAIL if pair == NPAIR - 1 else CHUNKS_HEAD
                    ogcol = 0
                    for ci, (bp0, nbp) in enumerate(chunks):
                        small = nbp == 5
                        xt = (x5pool if small else x8pool).tile(
                            [128, nbp * BANDW], BF,
                            tag="x5" if small else "x8", name="xt")
                        c0, c1 = bp0 * BANDW, (bp0 + nbp) * BANDW
                        nc.sync.dma_start(out=xt[0:K, :],
                                          in_=x[pair, 0:K, c0:c1])
                        nc.sync.dma_start(out=xt[64:64 + K, :],
                                          in_=x[pair, K:2 * K, c0:c1])
                        ogw = OG_5 if small else OG_8
                        og = (o5pool if small else o8pool).tile(
                            [M, ogw], BF,
                            tag="o5" if small else "o8", name="og")
                        goff = 0
                        for lb0, bsz in (BLOCKS_5 if small else BLOCKS_8):
                            tc.no_sync_barrier()
                            ps = pspool.tile([M, 2048], F32, tag="ps",
                                             name="ps")
                            for dj in range(KW):
                                for u in range(bsz):
                                    lv = lb0 + u
                                    xv = xt[:, lv * BANDW:
                                            (lv + 1) * BANDW].rearrange(
                                        "k (m w) -> k m w", m=2)
                                    for h in range(2):
                                        off = u * 1024 + h * 512
                                        nc.tensor.matmul(
                                            ps[:, off:off + NFREE],
                                            wt[h * 64:h * 64 + K,
                                               dj * M:(dj + 1) * M],
                                            xv[h * 64:h * 64 + K, :,
                                               dj:dj + OW],
                                            start=(dj == 0),
                                            stop=(dj == KW - 1),
                                        )
                            width = 2040 if bsz == 2 else 1016
                            nc.scalar.activation(
                                out=og[:, goff:goff + width],
                                in_=ps[:, 0:width],
                                func=mybir.ActivationFunctionType.Tanh,
                                bias=bt[:, 0:1],
                                scale=SCALE_IN,
                            )
                            goff += width
                        # split store: overlap output DMA, shrink tail
                        if pair == NPAIR - 1 and ci == len(chunks) - 1:
                            cuts = [0, 2040, 4080, ogw]
                        else:
                            cuts = [0, 4080, ogw]
                        for a, bnd in zip(cuts[:-1], cuts[1:]):
                            nc.gpsimd.dma_start(
                                out=y[pair][:, ogcol + a:ogcol + bnd],
                                in_=og[:, a:bnd])
                        ogcol += ogw

            for _ in range(iters):
                body()
    _split_excess_syncs(nc)
    return nc


_NC_CACHE = {}
LAST_EXEC_NS = None


def kernel(x: np.ndarray, W: np.ndarray, b: np.ndarray) -> np.ndarray:
    global LAST_EXEC_NS
    x = np.asarray(x, dtype=np.float32)
    W = np.asarray(W, dtype=np.float32)
    b = np.asarray(b, dtype=np.float32)

    wp = _pack_weights(W * _mask()).astype(NP_BF)
    bm = _pack_bias(b)
    xs = x.reshape(N_CORES, PER, CIN, H, W_IMG)

    iters = int(os.environ.get("KERNEL_ITERS", "1"))
    if iters not in _NC_CACHE:
        _NC_CACHE[iters] = _build_nc(iters)
    nc = _NC_CACHE[iters]

    in_maps = [
        {"x": _pack_x(xs[i], dtype=NP_BF), "wm": wp, "bm": bm}
        for i in range(N_CORES)
    ]
    res = run_bass_kernel_spmd(nc, in_maps, list(range(N_CORES)))
    LAST_EXEC_NS = res.exec_time_ns
    y = np.empty((B, COUT, OH, OW), dtype=np.float32)
    for i in range(N_CORES):
        _unpack_y_into(
            np.asarray(res.results[i]["y"], dtype=np.float32),
            y[i * PER:(i + 1) * PER],
        )
    y *= np.float32(SCALE_OUT)
    return y


# revision 14
# speedup vs baseline: 1.7001x; 1.0168x over previous
"""nn_C3PartialConv — v7.1: row-tiled K=60 band pairs, no q-duplication.

Banded Toeplitz formulation with PE row-tiling instead of column-shift
(q) folding:
  - Band = 6 output rows (needs 10 input rows x 6 cin = K 60).
  - Two bands run CONCURRENTLY as row tiles of the 128x128 PE array:
    even band on partitions 0-59 (tile_position (0,0)), odd band on
    partitions 64-123 (tile_position (64,0)).  One XBUS column carries
    both streams on disjoint partition lanes.
  - The 5 kernel columns dj are 5 accumulating matmuls per band with a
    free-dim window shift (252 + 4 = 256, so no pad column needed).
  - M = 96 = 6 out rows x 16 cout; N = 504 = 2 images x 252 cols.
  - x HBM traffic halves vs the q-folded v6 (no duplicated shift copy);
    x loads are chunked (5/8/8 band pairs) so the first matmul waits
    only on the first chunk, and only ONE odd 1-band-pair PSUM block
    per pair (block shorter than the ACT it waits on => PE stall).
  - tanh+bias applied by ScalarE over FOUR PSUM banks per instruction
    ([96, 2040], bands padded to 512 f32 = one bank each) to amortize
    the ~352-cycle ACT overhead; the 8 garbage columns between bands
    are dropped on the host.
  - og stores are split in two per chunk to overlap output DMA and
    shrink the tail.
"""

import os
import numpy as np
import ml_dtypes

import concourse.bass as bass
import concourse.tile as tile
from concourse import mybir
from concourse.bass_utils import run_bass_kernel_spmd

C3_CONNECTIONS = [
    [0, 1, 2], [1, 2, 3], [2, 3, 4], [3, 4, 5], [4, 5, 0], [5, 0, 1],
    [0, 1, 2, 3], [1, 2, 3, 4], [2, 3, 4, 5], [3, 4, 5, 0], [4, 5, 0, 1],
    [5, 0, 1, 2], [0, 1, 3, 4], [1, 2, 4, 5], [0, 2, 3, 5],
    [0, 1, 2, 3, 4, 5],
]

B, CIN, H, W_IMG = 64, 6, 256, 256
COUT, KH, KW = 16, 5, 5
OH = OW = 252
N_CORES = 8
PER = B // N_CORES          # 8 images per core
NPAIR = PER // 2            # 4 image pairs per core
G, S = 6, 10                # out rows / in rows per band
K, M = S * CIN, G * COUT    # 60, 96
NFREE = 2 * OW              # 504
NB = 42                     # bands per image pair (42*6 = 252)
NBP = NB // 2               # 21 band pairs
BANDW = 2 * W_IMG           # 512 free elems per (s,c) band row

# x chunks per pair: (bp start, n bp).  Pairs 0-2 lead with the small
# chunk so the pipeline fills fast; the last pair ends with it so the
# final og store (and final ACT) are small => short tail.
CHUNKS_HEAD = [(0, 5), (5, 8), (13, 8)]
CHUNKS_TAIL = [(0, 8), (8, 8), (16, 5)]
# psum blocks per chunk size: (local bp start, n bp)
BLOCKS_5 = [(0, 2), (2, 2), (4, 1)]
BLOCKS_8 = [(0, 2), (2, 2), (4, 2), (6, 2)]
OG_5 = 2 * 2040 + 1016      # 5096
OG_8 = 4 * 2040             # 8160
OG_PAIR = OG_5 + 2 * OG_8   # 21416 og cols per pair

BF = mybir.dt.bfloat16
F32 = mybir.dt.float32
NP_BF = ml_dtypes.bfloat16

SCALE_IN = 2.0 / 3.0
SCALE_OUT = 1.7159


def _mask() -> np.ndarray:
    m = np.zeros((COUT, CIN, KH, KW), dtype=np.float32)
    for i, conn in enumerate(C3_CONNECTIONS):
        m[i, conn] = 1.0
    return m


def _pack_weights(Wm: np.ndarray) -> np.ndarray:
    """[16,6,5,5] -> [128, 5*96].
    row p = s*6 + c (and 64+p duplicate); col = dj*96 + i*16 + o;
    value Wm[o, c, s-i, dj] for 0 <= s-i < 5."""
    wp = np.zeros((64, KW, G, COUT), dtype=np.float32)
    for i in range(G):
        for di in range(KH):
            s = i + di
            for c in range(CIN):
                wp[s * CIN + c, :, i, :] = Wm[:, c, di, :].T
    out = np.zeros((128, KW * M), dtype=np.float32)
    out[0:64] = wp.reshape(64, KW * M)
    out[64:128] = out[0:64]
    return out


def _pack_bias(b: np.ndarray) -> np.ndarray:
    """[16] -> [M, 1]: (2/3)*b[o] at partition i*16+o."""
    bm = np.empty((G, COUT), dtype=np.float32)
    bm[:] = SCALE_IN * b[None, :]
    return bm.reshape(M, 1)


def _pack_x(xs_core: np.ndarray, dtype) -> np.ndarray:
    """[PER,6,256,256] -> [NPAIR, 120, NBP*512].

    Row p = s*6+c holds even-band rows (image row 12j+s), row 60+p holds
    odd-band rows (image row 12j+6+s); free = j*512 + m*256 + w.  On
    device the halves land on SBUF partitions 0-59 / 64-123.
    """
    xp = xs_core.reshape(NPAIR, 2, CIN, H, W_IMG)
    st = xp.strides
    # full[n, m, c, j, t, w] = xp[n, m, c, 12j + t, w],  t in 0..15
    full = np.lib.stride_tricks.as_strided(
        xp,
        shape=(NPAIR, 2, CIN, NBP, 16, W_IMG),
        strides=(st[0], st[1], st[2], 12 * st[3], st[3], st[4]),
    )
    out = np.zeros((NPAIR, 128, NBP, 2, W_IMG), dtype=dtype)
    # -> [n, s, c, j, m, w]
    out[:, 0:K] = full[:, :, :, :, 0:S].transpose(0, 4, 2, 3, 1, 5).reshape(
        NPAIR, K, NBP, 2, W_IMG)
    out[:, 64:64 + K] = full[:, :, :, :, 6:6 + S].transpose(
        0, 4, 2, 3, 1, 5).reshape(NPAIR, K, NBP, 2, W_IMG)
    return out.reshape(NPAIR, 128, NBP * BANDW)


# og column segments per pair: (col, n bands); bands are 504 valid cols
# at 512-col stride within a segment; segments cover bands in order.
def _og_segments(chunks):
    segs = []
    col = 0
    for _, nbp in chunks:
        blocks = BLOCKS_5 if nbp == 5 else BLOCKS_8
        for _, bsz in blocks:
            segs.append((col, 2 * bsz))
            col += 2040 if bsz == 2 else 1016
    assert col == OG_PAIR
    return segs


OG_SEGS_HEAD = _og_segments(CHUNKS_HEAD)
OG_SEGS_TAIL = _og_segments(CHUNKS_TAIL)


def _unpack_y_into(y_dev: np.ndarray, out: np.ndarray) -> None:
    """[NPAIR, 96, OG_PAIR] -> out [PER,16,252,252] (unscaled)."""
    bands = np.empty((NPAIR, M, NB, NFREE), dtype=y_dev.dtype)
    st = y_dev.strides
    for n in range(NPAIR):
        segs = OG_SEGS_TAIL if n == NPAIR - 1 else OG_SEGS_HEAD
        b = 0
        for col, nb in segs:
            seg = np.lib.stride_tricks.as_strided(
                y_dev[n, :, col:],
                shape=(M, nb, NFREE),
                strides=(st[1], 512 * st[2], st[2]),
            )
            bands[n, :, b:b + nb] = seg
            b += nb
    # [n, i, o, b, m, w] -> [n, m, o, (b, i), w]
    yd = bands.reshape(NPAIR, G, COUT, NB, 2, OW).transpose(0, 4, 2, 3, 1, 5)
    out.reshape(NPAIR, 2, COUT, NB * G, OW)[...] = yd.reshape(
        NPAIR, 2, COUT, NB * G, OW)


def _split_excess_syncs(nc):
    def budget(ins):
        return 1 if isinstance(ins, (mybir.InstDrain, mybir.InstNoOp)) else 2

    for bb in nc.m.functions[0].blocks:
        new_insts = []
        for ins in bb.instructions:
            si = ins.sync_info
            w = list(si.on_wait) if si and si.on_wait else []
            u = list(si.on_update) if si and si.on_update else []
            cap = budget(ins)
            if len(w) + len(u) > cap:
                keep_n = max(0, cap - len(u))
                excess, kept = w[: len(w) - keep_n], w[len(w) - keep_n:]
                for wait in excess:
                    new_insts.append(
                        mybir.InstNoOp(
                            name=nc.get_next_instruction_name(),
                            sync_info=mybir.SyncInfo(on_wait=[wait],
                                                     on_update=[]),
                            bass_nofuse=True,
                            engine=ins.engine,
                        )
                    )
                ins.sync_info = mybir.SyncInfo(on_wait=kept, on_update=u)
            new_insts.append(ins)
        bb.instructions[:] = new_insts


def _build_nc(iters: int = 1):
    nc = bass.Bass()
    x = nc.declare_dram_parameter("x", [NPAIR, 128, NBP * BANDW], BF,
                                  isOutput=False)
    wm = nc.declare_dram_parameter("wm", [128, KW * M], BF, isOutput=False)
    bm = nc.declare_dram_parameter("bm", [M, 1], F32, isOutput=False)
    y = nc.declare_dram_parameter("y", [NPAIR, M, OG_PAIR], BF,
                                  isOutput=True)

    with tile.TileContext(nc) as tc:
        with (
            tc.tile_pool(name="consts", bufs=1) as consts,
            tc.tile_pool(name="x5", bufs=2) as x5pool,
            tc.tile_pool(name="x8", bufs=3) as x8pool,
            tc.tile_pool(name="ps", bufs=2, space="PSUM") as pspool,
            tc.tile_pool(name="o5", bufs=2) as o5pool,
            tc.tile_pool(name="o8", bufs=2) as o8pool,
        ):
            wt = consts.tile([128, KW * M], BF)
            nc.sync.dma_start(out=wt[:, :], in_=wm[:, :])
            bt = consts.tile([M, 1], F32)
            nc.sync.dma_start(out=bt[:, :], in_=bm[:, :])
            # preload the tanh table set
            warm = consts.tile([1, 1], F32)
            nc.scalar.activation(out=warm[:, :], in_=bt[0:1, :],
                                 func=mybir.ActivationFunctionType.Tanh)
            # HAM warm-up: ~3.4us of tiny matmuls during DMA-queue spin-up;
            # memset-sourced so they need no DMA at all.
            wmt = consts.tile([K, 128], BF)
            nc.gpsimd.memset(wmt[:, :], 0.25)
            wps = pspool.tile([M, 2048], F32, tag="ps", name="wps")
            for _ in range(32):
                nc.tensor.matmul(wps[0:16, 0:128], wmt[:, 0:16],
                                 wmt[:, :], start=True, stop=True)

            def body(_iv=None):
                for pair in range(NPAIR):
                    chunks = CHUNKS_TAIL if pair == NPAIR - 1 else CHUNKS_HEAD
                    ogcol = 0
                    for ci, (bp0, nbp) in enumerate(chunks):
                        small = nbp == 5
                        xt = (x5pool if small else x8pool).tile(
                            [128, nbp * BANDW], BF,
                            tag="x5" if small else "x8", name="xt")
                        c0, c1 = bp0 * BANDW, (bp0 + nbp) * BANDW
                        nc.sync.dma_start(out=xt[:, :],
                                          in_=x[pair, :, c0:c1])
                        ogw = OG_5 if small else OG_8
                        og = (o5pool if small else o8pool).tile(
                            [M, ogw], BF,
                            tag="o5" if small else "o8", name="og")
                        goff = 0
                        for lb0, bsz in (BLOCKS_5 if small else BLOCKS_8):
                            tc.no_sync_barrier()
                            ps = pspool.tile([M, 2048], F32, tag="ps",
                                             name="ps")
                            for dj in range(KW):
                                for u in range(bsz):
                                    lv = lb0 + u
                                    xv = xt[:, lv * BANDW:
                                            (lv + 1) * BANDW].rearrange(
                                        "k (m w) -> k m w", m=2)
                                    for h in range(2):
                                        off = u * 1024 + h * 512
                                        nc.tensor.matmul(
                                            ps[:, off:off + NFREE],
                                            wt[h * 64:h * 64 + K,
                                               dj * M:(dj + 1) * M],
                                            xv[h * 64:h * 64 + K, :,
                                               dj:dj + OW],
                                            start=(dj == 0),
                                            stop=(dj == KW - 1),
                                        )
                            width = 2040 if bsz == 2 else 1016
                            nc.scalar.activation(
                                out=og[:, goff:goff + width],
                                in_=ps[:, 0:width],
                                func=mybir.ActivationFunctionType.Tanh,
                                bias=bt[:, 0:1],
                                scale=SCALE_IN,
                            )
                            goff += width
                        # split store: overlap output DMA, shrink tail
                        if pair == NPAIR - 1 and ci == len(chunks) - 1:
                            cuts = [0, 2040, 4080, ogw]
                        else:
                            cuts = [0, 4080, ogw]
                        for a, bnd in zip(cuts[:-1], cuts[1:]):
                            nc.gpsimd.dma_start(
                                out=y[pair][:, ogcol + a:ogcol + bnd],
                                in_=og[:, a:bnd])
                        ogcol += ogw

            for _ in range(iters):
                body()
    _split_excess_syncs(nc)
    return nc


_NC_CACHE = {}
LAST_EXEC_NS = None


def kernel(x: np.ndarray, W: np.ndarray, b: np.ndarray) -> np.ndarray:
    global LAST_EXEC_NS
    x = np.asarray(x, dtype=np.float32)
    W = np.asarray(W, dtype=np.float32)
    b = np.asarray(b, dtype=np.float32)

    wp = _pack_weights(W * _mask()).astype(NP_BF)
    bm = _pack_bias(b)
    xs = x.reshape(N_CORES, PER, CIN, H, W_IMG)

    iters = int(os.environ.get("KERNEL_ITERS", "1"))
    if iters not in _NC_CACHE:
        _NC_CACHE[iters] = _build_nc(iters)
    nc = _NC_CACHE[iters]

    in_maps = [
        {"x": _pack_x(xs[i], dtype=NP_BF), "wm": wp, "bm": bm}
        for i in range(N_CORES)
    ]
    res = run_bass_kernel_spmd(nc, in_maps, list(range(N_CORES)))
    LAST_EXEC_NS = res.exec_time_ns
    y = np.empty((B, COUT, OH, OW), dtype=np.float32)
    for i in range(N_CORES):
        _unpack_y_into(
            np.asarray(res.results[i]["y"], dtype=np.float32),
            y[i * PER:(i + 1) * PER],
        )
    y *= np.float32(SCALE_OUT)
    return y
